# revision 1
# baseline (speedup 1.0000x reference)
"""Trainium2 Bass kernel for a 6-layer geometry-constrained cross-attention decoder.

Sharding: pure data-parallel over batch B=8 -> one batch element per NeuronCore.
Per-core layouts are feature-major ("T" = transposed): activations live as
[feature, token] so every matmul streams with full 128-partition contraction
and biases are per-partition. Attention probabilities are computed unnormalized
(exp with no max-subtraction; scores are O(1) by construction), masked by a
multiplicative {0,1} bf16 mask on the vector engine, and normalized after the
AV matmul via a ones-column appended to V (row 32 of the AV accumulator is the
softmax denominator).

Everything bf16 except: the residual stream, layernorm statistics, biases and
PSUM accumulation (all fp32). Measured vs fp64 reference: ~1.8e-3 max rel err.
"""

import os
import sys

for _p in ("/opt/trn_rl_repo", "/root/.axon_site/_ro/trn_rl_repo"):
    if os.path.isdir(_p) and _p not in sys.path:
        sys.path.insert(0, _p)

import numpy as np
import ml_dtypes

import concourse.bass as bass
import concourse.tile as tile
from concourse import bacc
from concourse import mybir
from concourse import bass_utils

BF16 = ml_dtypes.bfloat16
F32 = np.float32

B, NQ, NK, E, H, F, L = 8, 300, 4096, 256, 8, 2048, 6
D = E // H
SCALE = D ** -0.5
PC = 128          # partitions
EC = E // PC      # 2 feature chunks
FT = F // PC      # 16 ffn chunks
KT_CA = NK // PC  # 32 cross-attention key tiles
NKC = NK // 512   # 8 key column chunks for k-projection
TOK_TILES = [(0, 100), (100, 100), (200, 100)]   # 300 tokens, uniform
G_EXP = 3         # k-tiles per exp batch

dt = mybir.dt
Alu = mybir.AluOpType
Act = mybir.ActivationFunctionType

# smalls column map (per-partition fp32 vectors, feature f = 128*c + p)
C_BQK = 0     # 4 cols: sa qk bias (q: 0:2, k: 2:4)
C_BO_SA = 4   # 2
C_BQ_CA = 6   # 2
C_BK_CA = 8   # 2
C_BO_CA = 10  # 2
C_B1 = 12     # 16
C_B2 = 28     # 2
C_LN = 30     # 12: ln1g ln1b ln2g ln2b ln3g ln3b (2 each)
NS = 42


def _bcmid(ap2d, c):
    """[P, N] AP -> [P, c, N] with the middle dim broadcast (step 0)."""
    return bass.AP(tensor=ap2d.tensor, offset=ap2d.offset,
                   ap=[list(ap2d.ap[0]), [0, c], list(ap2d.ap[-1])])


def _hd(ap):
    """reshape trailing 256-wide feature dim into [8, 32] head/dim"""
    return ap.rearrange("p (h d) -> p h d", d=D)


def build_nc(nlayers=L):
    nc = bacc.Bacc("TRN2", target_bir_lowering=False, debug=False)
    f32, bf, f32r = dt.float32, dt.bfloat16, dt.float32r

    def din(name, shape, d=bf):
        return nc.dram_tensor(name, shape, d, kind="ExternalInput").ap()

    d_tT = din("tT", [E, NQ], f32)
    d_memT = din("memT", [E, NK])
    d_maskT = din("maskT", [NK, NQ])
    d_wqk = din("w_sa_qk", [nlayers, E, 2 * E])
    d_wsv = din("w_sa_v", [nlayers, E, E])
    d_wso = din("w_sa_o", [nlayers, E, E])
    d_wcq = din("w_ca_q", [nlayers, E, E])
    d_wck = din("w_ca_k", [nlayers, E, E])
    d_wcv = din("w_ca_v", [nlayers, E, E])
    d_wco = din("w_ca_o", [nlayers, E, E])
    d_w1 = din("w_f1", [nlayers, E, F])
    d_w2 = din("w_f2", [nlayers, F, E])
    d_sm = din("smalls", [nlayers, PC, NS], f32)
    d_vrow = din("vrow", [nlayers, PC, 2, E], f32)
    d_fin = din("finals", [PC, 4], f32)
    d_out = nc.dram_tensor("outT", [E, NQ], f32, kind="ExternalOutput").ap()

    def r2(ap):  # [256, X] -> [128, 2, X]
        return ap.rearrange("(c p) o -> p c o", p=PC)

    with tile.TileContext(nc) as tc:
        with (
            tc.tile_pool(name="persist", bufs=1) as pst,
            tc.tile_pool(name="wts", bufs=2) as wp,
            tc.tile_pool(name="acts", bufs=2) as acts,
            tc.tile_pool(name="probs", bufs=6) as probs,
            tc.tile_pool(name="stats", bufs=2) as stp,
            tc.tile_pool(name="ps_sc", bufs=2, space="PSUM") as ps_sc,
            tc.tile_pool(name="ps_pp", bufs=2, space="PSUM") as ps_pp,
        ):
            # ---- persistent loads ----
            memT = pst.tile([PC, EC, NK], bf, tag="memT", name="memT_sb")
            nc.sync.dma_start(out=memT, in_=r2(d_memT))
            maskT = pst.tile([PC, KT_CA, NQ], bf, tag="maskT", name="maskT_sb")
            nc.sync.dma_start(out=maskT, in_=d_maskT.rearrange("(t p) q -> p t q", p=PC))
            ones = pst.tile([PC, PC], bf, tag="ones", name="ones_sb")
            nc.vector.memset(ones, 1.0)
            eps = pst.tile([PC, 1], f32, tag="eps", name="eps_sb")
            nc.vector.memset(eps, 1e-5)
            fin = pst.tile([PC, 4], f32, tag="fin", name="fin_sb")
            nc.sync.dma_start(out=fin, in_=d_fin)
            vsa = pst.tile([PC, len(TOK_TILES), H, 2 * D], bf, tag="vsa", name="vsa_sb")
            nc.vector.memset(vsa[:, :, :, D:2 * D], 1.0)
            vca = pst.tile([PC, KT_CA, H, 2 * D], bf, tag="vca", name="vca_sb")
            nc.vector.memset(vca[:, :, :, D:2 * D], 1.0)

            tT = acts.tile([PC, EC, NQ], f32, tag="tT", name="tT0")
            nc.sync.dma_start(out=tT, in_=r2(d_tT))
            tb = acts.tile([PC, EC, NQ], bf, tag="tb", name="tb0")
            nc.gpsimd.tensor_copy(out=tb, in_=tT)

            def layernorm(l, r, gcol, name):
                """r: [128, 2, 300] f32 (+ gets normalized) -> new (tT, tb)"""
                rb = acts.tile([PC, EC, NQ], bf, tag="rb", name=f"rb{name}", bufs=1)
                nc.vector.tensor_copy(out=rb, in_=r)
                tsq = acts.tile([PC, EC, NQ], bf, tag="tsq", name=f"tsq{name}", bufs=1)
                nc.vector.tensor_mul(out=tsq, in0=rb, in1=rb)
                s0 = ps_pp.tile([PC, NQ], f32, tag="pp", name=f"lns0{name}")
                s1 = ps_pp.tile([PC, NQ], f32, tag="pp", name=f"lns1{name}")
                for c in range(EC):
                    nc.tensor.matmul(out=s0, lhsT=ones,
                                     rhs=rb[:, c, :],
                                     start=(c == 0), stop=(c == EC - 1))
                for c in range(EC):
                    nc.tensor.matmul(out=s1, lhsT=ones,
                                     rhs=tsq[:, c, :],
                                     start=(c == 0), stop=(c == EC - 1))
                mean = stp.tile([PC, NQ], f32, tag="mean", name=f"mean{name}", bufs=1)
                nc.vector.tensor_scalar_mul(out=mean, in0=s0, scalar1=1.0 / E)
                c1 = acts.tile([PC, EC, NQ], f32, tag="c1", name=f"c1{name}", bufs=1)
                nc.vector.tensor_sub(out=c1, in0=r, in1=_bcmid(mean, EC))
                msq = stp.tile([PC, NQ], f32, tag="msq", name=f"msq{name}", bufs=1)
                nc.vector.tensor_mul(out=msq, in0=mean, in1=mean)
                var = stp.tile([PC, NQ], f32, tag="var", name=f"var{name}", bufs=1)
                nc.vector.scalar_tensor_tensor(out=var, in0=s1, scalar=1.0 / E,
                                               in1=msq, op0=Alu.mult, op1=Alu.subtract)
                sd = stp.tile([PC, NQ], f32, tag="sd", name=f"sd{name}", bufs=1)
                nc.scalar.activation(out=sd, in_=var, func=Act.Sqrt, bias=eps[:, 0:1])
                rstd = stp.tile([PC, NQ], f32, tag="rstd", name=f"rstd{name}", bufs=1)
                nc.vector.reciprocal(out=rstd, in_=sd)
                c2 = acts.tile([PC, EC, NQ], f32, tag="c2", name=f"c2{name}", bufs=1)
                nc.vector.tensor_mul(out=c2, in0=c1, in1=_bcmid(rstd, EC))
                t_new = acts.tile([PC, EC, NQ], f32, tag="tT", name=f"t{name}")
                tb_new = acts.tile([PC, EC, NQ], bf, tag="tb", name=f"tb{name}")
                if gcol is None:
                    g, b = fin[:, 0:2], fin[:, 2:4]
                else:
                    g = sm[:, gcol:gcol + 2]
                    b = sm[:, gcol + 2:gcol + 4]
                for c in range(EC):
                    # tb (bf16, feeds the next matmuls -> critical path) on DVE;
                    # tT (fp32 residual, consumed later) on gpsimd
                    nc.vector.tensor_scalar(out=tb_new[:, c, :], in0=c2[:, c, :],
                                            scalar1=g[:, c:c + 1], scalar2=b[:, c:c + 1],
                                            op0=Alu.mult, op1=Alu.add)
                    nc.gpsimd.tensor_scalar(out=t_new[:, c, :], in0=c2[:, c, :],
                                            scalar1=g[:, c:c + 1], scalar2=b[:, c:c + 1],
                                            op0=Alu.mult, op1=Alu.add)
                return t_new, tb_new

            def attention(q_sb, k_s_b, v_sb, k_tiles, mask_sb, name, g_exp=G_EXP,
                          bg_split=None, split_g=9):
                k_sb = k_s_b
                """generic attention: q_sb/k_sb: [128, C, NQ/N] bf16 feature-major;
                v_sb: [128, ntile, H, 33]; returns attn [128, 2, 300] bf16"""
                attn = acts.tile([PC, EC, NQ], bf, tag=f"attn", name=f"attn{name}")
                nkt = len(k_tiles)
                for h in range(H):
                    po = 32 * (h % 4)
                    ci = h // 4
                    qh = q_sb[po:po + 32, ci, 0:NQ]
                    av = ps_pp.tile([PC, NQ], f32, tag="pp", name=f"av{name}h{h}")
                    deferred = []
                    g = 0
                    while g < nkt:
                        gsz = min(g_exp, nkt - g)
                        kg = k_tiles[g][1]  # uniform tile height in this group
                        assert all(k_tiles[g + j][1] == kg for j in range(gsz))
                        sc = ps_sc.tile([PC, G_EXP, 512], f32, tag="sc",
                                        name=f"sc{name}h{h}g{g}")
                        for j in range(gsz):
                            kt0, ksz = k_tiles[g + j]
                            nc.tensor.matmul(
                                out=sc[0:ksz, j, 0:NQ],
                                lhsT=k_sb[po:po + 32, ci, kt0:kt0 + ksz],
                                rhs=qh, start=True, stop=True,
                                tile_position=(po, 0))
                        p = probs.tile([PC, G_EXP, NQ], bf, tag="p",
                                       name=f"p{name}h{h}g{g}")
                        nc.scalar.activation(out=p[0:kg, 0:gsz, :],
                                             in_=sc[0:kg, 0:gsz, 0:NQ], func=Act.Exp)
                        if mask_sb is not None:
                            pm = probs.tile([PC, G_EXP, NQ], bf, tag="pm",
                                            name=f"pm{name}h{h}g{g}")
                            nc.vector.tensor_mul(out=pm[0:kg, 0:gsz, :],
                                                 in0=p[0:kg, 0:gsz, :],
                                                 in1=mask_sb[0:kg, g:g + gsz, :])
                        else:
                            pm = p
                        if h == 0 and bg_split is not None and g >= split_g:
                            deferred.append((g, gsz, pm))
                        else:
                            for j in range(gsz):
                                kt0, ksz = k_tiles[g + j]
                                ti = g + j
                                nc.tensor.matmul(
                                    out=av[0:2 * D, 0:NQ],
                                    lhsT=v_sb[0:ksz, ti, h, 0:2 * D],
                                    rhs=pm[0:ksz, j, 0:NQ],
                                    start=(ti == 0), stop=(ti == nkt - 1),
                                    tile_position=(0, 0))
                        g += gsz
                    if h == 0 and bg_split is not None:
                        bg_split()
                        for g_, gsz_, pm_ in deferred:
                            for j in range(gsz_):
                                kt0, ksz = k_tiles[g_ + j]
                                ti = g_ + j
                                nc.tensor.matmul(
                                    out=av[0:2 * D, 0:NQ],
                                    lhsT=v_sb[0:ksz, ti, h, 0:2 * D],
                                    rhs=pm_[0:ksz, j, 0:NQ],
                                    start=(ti == 0), stop=(ti == nkt - 1),
                                    tile_position=(0, 0))
                    recip = stp.tile([32, NQ], f32, tag="recip", name=f"rc{name}h{h}", bufs=4)
                    nc.vector.reciprocal(out=recip, in_=av[D:2 * D, 0:NQ])
                    nc.vector.tensor_mul(out=attn[po:po + 32, ci, :],
                                         in0=av[0:32, 0:NQ], in1=recip)
                return attn

            def out_proj_residual(l, w_sb, attn, bcol, tT, name):
                r = acts.tile([PC, EC, NQ], f32, tag="r", name=f"r{name}", bufs=1)
                for co in range(EC):
                    po = ps_pp.tile([PC, NQ], f32, tag="pp", name=f"po{name}{co}")
                    for ci in range(EC):
                        nc.tensor.matmul(out=po, lhsT=w_sb[:, ci, PC * co:PC * (co + 1)],
                                         rhs=attn[:, ci, :],
                                         start=(ci == 0), stop=(ci == EC - 1))
                    nc.vector.scalar_tensor_tensor(
                        out=r[:, co, :], in0=po, scalar=sm[:, bcol + co:bcol + co + 1],
                        in1=tT[:, co, :], op0=Alu.add, op1=Alu.add)
                return r

            def emit_kproj(l, wck, sm):
                kT = acts.tile([PC, EC, NK], bass.mybir.dt.bfloat16, tag="kT",
                               name=f"kT{l}", bufs=2)
                for co in range(EC):
                    for nk in range(NKC):
                        pk = ps_pp.tile([PC, 512], f32, tag="pp", name=f"pk{l}_{co}_{nk}")
                        for ci in range(EC):
                            nc.tensor.matmul(out=pk,
                                             lhsT=wck[:, ci, PC * co:PC * (co + 1)],
                                             rhs=memT[:, ci, 512 * nk:512 * (nk + 1)],
                                             start=(ci == 0), stop=(ci == EC - 1))
                        nc.vector.tensor_scalar(
                            out=kT[:, co, 512 * nk:512 * (nk + 1)], in0=pk,
                            scalar1=sm[:, C_BK_CA + co:C_BK_CA + co + 1],
                            scalar2=None, op0=Alu.add)
                return kT

            kT_next = None
            wsm_next = None
            for l in range(nlayers):
                # ---- layer weight loads ----
                wqk = wp.tile([PC, EC, 2 * E], bass.mybir.dt.bfloat16, tag="wqk", name=f"wqk{l}")
                nc.sync.dma_start(out=wqk, in_=r2(d_wqk[l]))
                wsv = wp.tile([PC, EC, E], bass.mybir.dt.bfloat16, tag="wsv", name=f"wsv{l}")
                nc.sync.dma_start(out=wsv, in_=r2(d_wsv[l]))
                wso = wp.tile([PC, EC, E], bass.mybir.dt.bfloat16, tag="wso", name=f"wso{l}")
                nc.sync.dma_start(out=wso, in_=r2(d_wso[l]))
                wcq = wp.tile([PC, EC, E], bass.mybir.dt.bfloat16, tag="wcq", name=f"wcq{l}")
                nc.sync.dma_start(out=wcq, in_=r2(d_wcq[l]))
                if l == 0:
                    wck = wp.tile([PC, EC, E], bass.mybir.dt.bfloat16, tag="wck", name=f"wck{l}")
                    nc.sync.dma_start(out=wck, in_=r2(d_wck[l]))
                wcv = wp.tile([PC, EC, E], bass.mybir.dt.bfloat16, tag="wcv", name=f"wcv{l}")
                nc.sync.dma_start(out=wcv, in_=r2(d_wcv[l]))
                wco = wp.tile([PC, EC, E], bass.mybir.dt.bfloat16, tag="wco", name=f"wco{l}")
                nc.sync.dma_start(out=wco, in_=r2(d_wco[l]))
                w1 = wp.tile([PC, EC, F], bass.mybir.dt.bfloat16, tag="w1", name=f"w1_{l}", bufs=1)
                nc.sync.dma_start(out=w1, in_=r2(d_w1[l]))
                w2 = wp.tile([PC, FT, E], bass.mybir.dt.bfloat16, tag="w2", name=f"w2_{l}", bufs=1)
                nc.sync.dma_start(out=w2, in_=d_w2[l].rearrange("(c p) o -> p c o", p=PC))
                if l == 0:
                    sm = wp.tile([PC, NS], f32, tag="sm", name=f"sm{l}")
                    nc.sync.dma_start(out=sm, in_=d_sm[l])
                else:
                    sm = wsm_next
                vrow = wp.tile([PC, 2, E], f32, tag="vrow", name=f"vrow{l}")
                nc.sync.dma_start(out=vrow, in_=d_vrow[l])

                # ---- SA qkv projections ----
                qk_sa = acts.tile([PC, 4, NQ], bass.mybir.dt.bfloat16, tag="qk_sa", name=f"qk_sa{l}")
                if True:
                    for co in range(4):
                        po = ps_pp.tile([PC, NQ], f32, tag="pp", name=f"pqk{l}_{co}")
                        for ci in range(EC):
                            nc.tensor.matmul(out=po, lhsT=wqk[:, ci, PC * co:PC * (co + 1)],
                                             rhs=tb[:, ci, :],
                                             start=(ci == 0), stop=(ci == EC - 1))
                        # q gets the attention scale folded in
                        nc.vector.tensor_scalar(
                            out=qk_sa[:, co, :], in0=po,
                            scalar1=sm[:, C_BQK + co:C_BQK + co + 1],
                            scalar2=SCALE if co < 2 else 1.0,
                            op0=Alu.add, op1=Alu.mult)
                    for tt, (t0, tsz) in enumerate(TOK_TILES):
                        pv = ps_pp.tile([PC, E], f32, tag="pp", name=f"pvsa{l}_{tt}")
                        for ci in range(EC):
                            nc.tensor.matmul(out=pv[0:tsz, :],
                                             lhsT=tb[:, ci, t0:t0 + tsz],
                                             rhs=wsv[:, ci, :],
                                             start=(ci == 0), stop=(ci == EC - 1))
                        nc.vector.tensor_add(
                            out=vsa[0:tsz, tt, :, 0:D],
                            in0=_hd(pv[0:tsz, :]),
                            in1=_hd(vrow[0:tsz, 0, :]))

                # ---- SA attention ----
                attn = attention(qk_sa[:, 0:2, :], qk_sa[:, 2:4, :],
                                 vsa, TOK_TILES, None, f"sa{l}", g_exp=3)
                # CA v-projection hoisted here: depends only on memT/wcv, and
                # the WAR on vca (prev layer's CA attention) is already clear.


                # ---- SA out proj + LN1 ----
                r = out_proj_residual(l, wso, attn, C_BO_SA, tT, f"so{l}")
                tT, tb = layernorm(l, r, C_LN, f"ln1_{l}")

                # ---- CA projections ----
                q_ca = acts.tile([PC, EC, NQ], bass.mybir.dt.bfloat16, tag="q_ca", name=f"q_ca{l}")
                kT = kT_next if kT_next is not None else emit_kproj(l, wck, sm)
                kT_next = None
                if True:
                    for co in range(EC):
                        po = ps_pp.tile([PC, NQ], f32, tag="pp", name=f"pq_ca{l}_{co}")
                        for ci in range(EC):
                            nc.tensor.matmul(out=po, lhsT=wcq[:, ci, PC * co:PC * (co + 1)],
                                             rhs=tb[:, ci, :],
                                             start=(ci == 0), stop=(ci == EC - 1))
                        nc.vector.tensor_scalar(
                            out=q_ca[:, co, :], in0=po,
                            scalar1=sm[:, C_BQ_CA + co:C_BQ_CA + co + 1],
                            scalar2=SCALE, op0=Alu.add, op1=Alu.mult)
                    def emit_vproj(lo, hi, l=l, wcv=wcv, vrow=vrow):
                        for tt in range(lo, hi):
                            pv = ps_pp.tile([PC, E], f32, tag="pp", name=f"pvca{l}_{tt}")
                            for ci in range(EC):
                                nc.tensor.matmul(out=pv,
                                                 lhsT=memT[:, ci, PC * tt:PC * (tt + 1)],
                                                 rhs=wcv[:, ci, :],
                                                 start=(ci == 0), stop=(ci == EC - 1))
                            nc.vector.tensor_add(
                                out=vca[:, tt, :, 0:D],
                                in0=_hd(pv),
                                in1=_hd(vrow[:, 1, :]))
                    emit_vproj(0, 12)

                # ---- CA attention ----
                ca_tiles = [(PC * i, PC) for i in range(KT_CA)]
                attn = attention(q_ca, kT, vca, ca_tiles, maskT, f"ca{l}",
                                 bg_split=lambda: emit_vproj(12, KT_CA))
                if l + 1 < nlayers:
                    wck_n = wp.tile([PC, EC, E], bass.mybir.dt.bfloat16, tag="wck", name=f"wck{l + 1}")
                    nc.sync.dma_start(out=wck_n, in_=r2(d_wck[l + 1]))
                    sm_n = wp.tile([PC, NS], f32, tag="sm", name=f"sm{l + 1}")
                    nc.sync.dma_start(out=sm_n, in_=d_sm[l + 1])
                    kT_next = emit_kproj(l + 1, wck_n, sm_n)
                    wsm_next = sm_n

                # ---- CA out proj + LN2 ----
                r = out_proj_residual(l, wco, attn, C_BO_CA, tT, f"co{l}")
                tT, tb = layernorm(l, r, C_LN + 4, f"ln2_{l}")

                # ---- FFN ----
                hT = acts.tile([PC, FT, NQ], bass.mybir.dt.bfloat16, tag="hT", name=f"hT{l}", bufs=1)
                if True:
                    for ft in range(FT):
                        pf = ps_pp.tile([PC, NQ], f32, tag="pp", name=f"pf1_{l}_{ft}")
                        for ci in range(EC):
                            nc.tensor.matmul(out=pf, lhsT=w1[:, ci, PC * ft:PC * (ft + 1)],
                                             rhs=tb[:, ci, :],
                                             start=(ci == 0), stop=(ci == EC - 1))
                        if ft % 2 == 0:
                            nc.scalar.activation(out=hT[:, ft, :], in_=pf, func=Act.Relu,
                                                 bias=sm[:, C_B1 + ft:C_B1 + ft + 1])
                        else:
                            nc.vector.tensor_scalar(
                                out=hT[:, ft, :], in0=pf,
                                scalar1=sm[:, C_B1 + ft:C_B1 + ft + 1], scalar2=0.0,
                                op0=Alu.add, op1=Alu.max)
                    r = acts.tile([PC, EC, NQ], f32, tag="r", name=f"rf{l}", bufs=1)
                    for co in range(EC):
                        p2 = ps_pp.tile([PC, NQ], f32, tag="pp", name=f"pf2_{l}_{co}")
                        for fc in range(FT):
                            nc.tensor.matmul(out=p2, lhsT=w2[:, fc, PC * co:PC * (co + 1)],
                                             rhs=hT[:, fc, :],
                                             start=(fc == 0), stop=(fc == FT - 1))
                        nc.vector.scalar_tensor_tensor(
                            out=r[:, co, :], in0=p2,
                            scalar=sm[:, C_B2 + co:C_B2 + co + 1],
                            in1=tT[:, co, :], op0=Alu.add, op1=Alu.add)
                tT, tb = layernorm(l, r, C_LN + 8, f"ln3_{l}")

            # ---- final LN + store ----
            outT, _ = layernorm(None, tT, None, "lnf")
            nc.sync.dma_start(out=r2(d_out), in_=outT)

    nc.compile()
    return nc


def _pack_inputs(inputs, nlayers=L):
    """Host-side layout prep: transpose / cast / pack. Returns per-core in_maps."""
    bf = BF16
    smalls = np.zeros((nlayers, PC, NS), np.float32)
    vrow = np.zeros((nlayers, PC, 2, E), np.float32)
    for l in range(nlayers):
        def put(col, vec):
            n = vec.shape[0] // PC
            smalls[l, :, col:col + n] = vec.reshape(n, PC).T
        put(C_BQK, np.asarray(inputs["sa_bqkv"][l][:2 * E], np.float32))
        put(C_BO_SA, np.asarray(inputs["sa_bo"][l], np.float32))
        put(C_BQ_CA, np.asarray(inputs["ca_bq"][l], np.float32))
        put(C_BK_CA, np.asarray(inputs["ca_bk"][l], np.float32))
        put(C_BO_CA, np.asarray(inputs["ca_bo"][l], np.float32))
        put(C_B1, np.asarray(inputs["f_b1"][l], np.float32))
        put(C_B2, np.asarray(inputs["f_b2"][l], np.float32))
        put(C_LN, np.asarray(inputs["ln1g"][l], np.float32))
        put(C_LN + 2, np.asarray(inputs["ln1b"][l], np.float32))
        put(C_LN + 4, np.asarray(inputs["ln2g"][l], np.float32))
        put(C_LN + 6, np.asarray(inputs["ln2b"][l], np.float32))
        put(C_LN + 8, np.asarray(inputs["ln3g"][l], np.float32))
        put(C_LN + 10, np.asarray(inputs["ln3b"][l], np.float32))
        vrow[l, :, 0, :] = np.asarray(inputs["sa_bqkv"][l][2 * E:], np.float32)[None, :]
        vrow[l, :, 1, :] = np.asarray(inputs["ca_bv"][l], np.float32)[None, :]
    finals = np.zeros((PC, 4), np.float32)
    finals[:, 0:2] = np.asarray(inputs["lnfg"], np.float32).reshape(2, PC).T
    finals[:, 2:4] = np.asarray(inputs["lnfb"], np.float32).reshape(2, PC).T

    def T(x):
        return np.ascontiguousarray(np.swapaxes(np.asarray(x), -1, -2))

    shared = {
        "w_sa_qk": T(inputs["sa_wqkv"][:nlayers, :2 * E]).astype(bf),
        "w_sa_v": T(inputs["sa_wqkv"][:nlayers, 2 * E:]).astype(bf),
        "w_sa_o": T(inputs["sa_wo"][:nlayers]).astype(bf),
        "w_ca_q": T(inputs["ca_wq"][:nlayers]).astype(bf),
        "w_ca_k": T(inputs["ca_wk"][:nlayers]).astype(bf),
        "w_ca_v": T(inputs["ca_wv"][:nlayers]).astype(bf),
        "w_ca_o": T(inputs["ca_wo"][:nlayers]).astype(bf),
        "w_f1": T(inputs["f_w1"][:nlayers]).astype(bf),
        "w_f2": T(inputs["f_w2"][:nlayers]).astype(bf),
        "smalls": smalls,
        "vrow": vrow,
        "finals": finals,
    }
    in_maps = []
    for b in range(B):
        m = dict(shared)
        m["tT"] = T(inputs["tgt"][b]).astype(np.float32)
        m["memT"] = T(inputs["memory"][b]).astype(bf)
        m["maskT"] = T(inputs["geometry_mask"][b]).astype(bf)
        in_maps.append(m)
    return in_maps


_CACHE = {}


def kernel(run_opts=None, **inputs):
    nlayers = L
    if "nc" not in _CACHE:
        _CACHE["nc"] = build_nc(nlayers)
    nc = _CACHE["nc"]
    in_maps = _pack_inputs(inputs, nlayers)
    res = bass_utils.run_bass_kernel_spmd(
        nc, in_maps, core_ids=list(range(B)), **(run_opts or {}))
    _CACHE["last_result"] = res
    out = np.stack([np.asarray(r["outT"]).T for r in res.results])
    return np.ascontiguousarray(out.astype(np.float32))



# revision 43
# speedup vs baseline: 1.1630x; 1.1630x over previous
"""Trainium2 Bass kernel for a 6-layer geometry-constrained cross-attention decoder.

Sharding: pure data-parallel over batch B=8 -> one batch element per NeuronCore.

v5 (hardware-legal): fp8 DoubleRow + mask folded into scores + 2-engine exp.
- All E=256-contraction projections (SA qkv, CA q/k/v) run as single fp8
  DoubleRow matmuls (contraction 2x128 in one instruction at 0.5 cycles/row).
- CA scores: DoubleRow pairs (-C*I @ maskInv) with (K_band @ q) so one
  instruction yields s - C*(1-mask); exp alone realizes the mask (no vector
  mask-multiply at all). K bands live in a zero-padded arena (bd) refreshed
  per layer by 8 strided SBUF->SBUF DMAs from the k-projection output.
- Only Act and DVE can read PSUM (GPSIMD cannot), so exp runs as k-tile-pair
  instructions on those two: AA pairs = exact exp -> fp8 probs (enables
  DoubleRow AV), DD pairs = Schraudolph int16-bitcast fast exp (~±3% on
  probabilities, washes out in the softmax average) -> bf16 probs.
- AV is flipped for both SA and CA: probs are the stationary operand, out is
  [q, head*(d+1)] accumulated in one psum bank per 4-head pass (the 33rd
  column of V is ones -> denominator); normalization is a per-(q,h)
  reciprocal + broadcast-mul, and an f32 PE transpose restores feature-major.
- PSUM map (8 banks): 3x [128,2,512] rotating pair-tiles (scores; also all
  projection / FFN / transpose outputs outside the attention windows) plus
  1x [128,2,512] aux tile (AV accumulators slot-per-pass, LN stats, kproj).
- Pool (gpsimd) takes all SBUF->SBUF elementwise work: LN rb/tsq/msq/c1/c2/
  t_new/tb, freeing Act/DVE for the PSUM-drain work only.
- k-bias dropped (cancels in softmax); v-bias folded into out-proj bias on
  the host; sqrt(SCALE) folded into q and k weights; fp8 weights pre-scaled
  by 16 to stay in e4m3 normal range, undone in the psum evacuation ops.
- FFN / out-projections / layernorm stay bf16/f32.
"""

import os
import sys

for _p in ("/opt/trn_rl_repo", "/root/.axon_site/_ro/trn_rl_repo"):
    if os.path.isdir(_p) and _p not in sys.path:
        sys.path.insert(0, _p)

import numpy as np
import ml_dtypes

import concourse.bass as bass
import concourse.tile as tile
from concourse import bacc
from concourse import mybir
from concourse import bass_utils

BF16 = ml_dtypes.bfloat16
F8 = ml_dtypes.float8_e4m3
F32 = np.float32

B, NQ, NK, E, H, F, L = 8, 300, 4096, 256, 8, 2048, 6
D = E // H
SCALE = D ** -0.5
RS = float(np.sqrt(SCALE))   # folded into q and k weights/biases
WS = 16.0                    # fp8 weight pre-scale (undone in psum evac)
CMASK = 11.0                 # additive mask penalty; exp(-11) ~ 1.7e-5 -> fp8 0
A16 = 128.0 / float(np.log(2.0))     # Schraudolph bf16 slope
B16 = 127.0 * 128.0 - 5.59           # Schraudolph bf16 bias (minimax-centred)
PC = 128          # partitions
EC = E // PC      # 2 feature chunks
FT = F // PC      # 16 ffn chunks
KT = NK // PC     # 32 cross-attention key tiles
NKC = NK // 512   # 8 key column chunks for k-projection
NQP = 304         # query stride padded so DoubleRow AP steps are %16==0
VW = 34           # value width: d + ones column (denominator) + pad (%16)
TOK_TILES = [(0, 100), (100, 100), (200, 100)]   # SA key tiles
QCH = [(0, 100), (100, 100), (200, 100)]         # query chunks for flipped AV

QOFF = KT * NQP   # q_ca offset inside the mask arena
BDH = PC * KT     # per-head band stride in bd arena
BDHALF = PC + 4 * BDH          # one arena half: mci column + 4 head bands


def _bd_mci(h):
    return (h // 4) * BDHALF


def _bd_band(h, t):
    # ISA AP steps are 16-bit, so each 4-head half has its own -C*I column;
    # max slot stride = PC + 3*BDH + 31*PC = 16384.
    return _bd_mci(h) + PC + (h % 4) * BDH + t * PC

dt = mybir.dt
Alu = mybir.AluOpType
Act = mybir.ActivationFunctionType
DR = mybir.MatmulPerfMode.DoubleRow

# smalls column map (per-partition fp32 vectors, feature f = 128*c + p)
C_BQ_SA = 0   # 2: sqrt(S)*sa q bias
C_BO_SA = 2   # 2: sa_bo + sa_wo @ sa_bv
C_BQ_CA = 4   # 2: sqrt(S)*ca_bq
C_BO_CA = 6   # 2: ca_bo + ca_wo @ ca_bv
C_B1 = 8      # 16
C_B2 = 24     # 2
C_LN = 26     # 12: ln1g ln1b ln2g ln2b ln3g ln3b (2 each)
NS = 38


def _spread(counts):
    """Deterministic balanced interleave, e.g. {'A':3,'D':2} -> ADADA."""
    out = []
    acc = {k: 0.0 for k in counts}
    total = sum(counts.values())
    for _ in range(total):
        for k in counts:
            acc[k] += counts[k] / total
        k = max(acc, key=lambda q: acc[q])
        acc[k] -= 1.0
        out.append(k)
    return out


# exp engine pair-classes: AA = exact fp8 on Act (enables DoubleRow AV);
# DD = Schraudolph bf16 on DVE.
import os as _os
CA_CLS = _spread({"AA": 33, "DD": 31}) if _os.environ.get("NOSCHRAU") != "1" else ["AA"]
SA_CLS = _spread({"AA": 1, "DD": 1}) if _os.environ.get("NOSCHRAU") != "1" else ["AA"]
SA_SGL = _spread({"A": 1, "D": 1}) if _os.environ.get("NOSCHRAU") != "1" else ["A"]
RELU_E = _spread({"A": 1, "D": 1})
EVAC_E = _spread({"A": 1, "D": 1})


def _bcmid(ap2d, c):
    """[P, N] AP -> [P, c, N] with the middle dim broadcast (step 0)."""
    return bass.AP(tensor=ap2d.tensor, offset=ap2d.offset,
                   ap=[list(ap2d.ap[0]), [0, c], list(ap2d.ap[-1])])


def _bclast(ap3, n):
    """[P, A, 1] AP -> [P, A, n] with the last dim broadcast (step 0)."""
    return bass.AP(tensor=ap3.tensor, offset=ap3.offset,
                   ap=[list(ap3.ap[0]), list(ap3.ap[1]), [0, n]])


def _slot2(arena, col0, col1, n):
    """[128, 2, n] DoubleRow slot AP over a 2D arena tile."""
    base = arena[:, col0:col0 + n]
    return bass.AP(tensor=base.tensor, offset=base.offset,
                   ap=[list(base.ap[0]), [col1 - col0, 2], [1, n]])


def _hd(ap):
    """reshape trailing 256-wide feature dim into [8, 32] head/dim"""
    return ap.rearrange("p (h d) -> p h d", d=D)


def build_nc(nlayers=L):
    nc = bacc.Bacc("TRN2", target_bir_lowering=False, debug=False)
    f32, bf, f8 = dt.float32, dt.bfloat16, dt.float8e4

    def din(name, shape, d=f8):
        return nc.dram_tensor(name, shape, d, kind="ExternalInput").ap()

    d_tT = din("tT", [E, NQ], f32)
    d_memT = din("memT8", [E, NK])
    d_maskI = din("maskI8", [PC, KT, NQ])
    d_bd0 = din("bd0", [PC, 2 * BDHALF])
    d_vca0 = din("vca0", [PC, KT, H, VW])
    d_vsa0 = din("vsa0", [PC, len(TOK_TILES), H, VW])
    d_idb = din("idb", [PC, PC], f32)
    d_wqk = din("w_sa_qk8", [nlayers, E, 2 * E])
    d_wsv = din("w_sa_v8", [nlayers, E, E])
    d_wso = din("w_sa_o", [nlayers, E, E], bf)
    d_wcq = din("w_ca_q8", [nlayers, E, E])
    d_wck = din("w_ca_k8", [nlayers, E, E])
    d_wcv = din("w_ca_v8", [nlayers, E, E])
    d_wco = din("w_ca_o", [nlayers, E, E], bf)
    d_w1 = din("w_f1", [nlayers, E, F], bf)
    d_w2 = din("w_f2", [nlayers, F, E], bf)
    d_sm = din("smalls", [nlayers, PC, NS], f32)
    d_fin = din("finals", [PC, 4], f32)
    d_out = nc.dram_tensor("outT", [E, NQ], f32, kind="ExternalOutput").ap()
    DBG = os.environ.get("KDBG") == "1"
    d_dbg = [nc.dram_tensor(f"dbg{i}", [E, NQ], f32, kind="ExternalOutput").ap()
             for i in range(3)] if DBG else None
    d_dba = nc.dram_tensor("dbattn", [PC, EC, NQ], bf, kind="ExternalOutput").ap()         if DBG else None
    d_dbq = nc.dram_tensor("dbqk", [PC, 4, NQ], f8, kind="ExternalOutput").ap()         if DBG else None
    d_dbv = nc.dram_tensor("dbvsa", [PC, len(TOK_TILES), H, VW], f8,
                           kind="ExternalOutput").ap() if DBG else None

    def r2(ap):  # [256, X] -> [128, 2, X]
        return ap.rearrange("(c p) o -> p c o", p=PC)

    with tile.TileContext(nc) as tc:
        with (
            tc.tile_pool(name="persist", bufs=1) as pst,
            tc.tile_pool(name="wts", bufs=2) as wp,
            tc.tile_pool(name="acts", bufs=2) as acts,
            tc.tile_pool(name="probs", bufs=12) as probs,
            tc.tile_pool(name="stats", bufs=2) as stp,
            tc.tile_pool(name="ps_sc", bufs=3, space="PSUM") as ps_sc,
            tc.tile_pool(name="ps_pp", bufs=1, space="PSUM") as ps_pp,
        ):
            # ---- persistent loads ----
            memT = pst.tile([PC, EC, NK], f8, tag="memT", name="memT_sb")
            nc.sync.dma_start(out=memT, in_=r2(d_memT))
            marena = pst.tile([PC, QOFF + EC * NQP], f8, tag="marena", name="marena_sb")
            nc.sync.dma_start(
                out=marena[:, 0:QOFF].rearrange("p (t q) -> p t q", q=NQP)[:, :, 0:NQ],
                in_=d_maskI)
            bd = pst.tile([PC, 2 * BDHALF], f8, tag="bd", name="bd_sb")
            nc.sync.dma_start(out=bd, in_=d_bd0)
            ones = pst.tile([PC, PC], bf, tag="ones", name="ones_sb")
            nc.vector.memset(ones, 1.0)
            idb = pst.tile([PC, PC], f32, tag="idb", name="idb_sb")
            nc.sync.dma_start(out=idb, in_=d_idb)
            eps = pst.tile([PC, 1], f32, tag="eps", name="eps_sb")
            nc.vector.memset(eps, 1e-5)
            fin = pst.tile([PC, 4], f32, tag="fin", name="fin_sb")
            nc.sync.dma_start(out=fin, in_=d_fin)
            vsa = pst.tile([PC, len(TOK_TILES), H, VW], f8, tag="vsa", name="vsa_sb")
            nc.sync.dma_start(out=vsa, in_=d_vsa0)
            vca = pst.tile([PC, KT, H, VW], f8, tag="vca", name="vca_sb")
            nc.sync.dma_start(out=vca, in_=d_vca0)
            kT = pst.tile([PC, EC, NK], f8, tag="kT", name="kT_sb")

            tT = acts.tile([PC, EC, NQ], f32, tag="tT", name="tT0")
            nc.sync.dma_start(out=tT, in_=r2(d_tT))
            tb8 = acts.tile([PC, EC, NQP], f8, tag="tb8", name="tb8_0")
            nc.gpsimd.tensor_copy(out=tb8[:, :, 0:NQ], in_=tT)

            evac_i = [0]

            def evac_eng():
                e = EVAC_E[evac_i[0] % len(EVAC_E)]
                evac_i[0] += 1
                return e

            def evac_scale(out_ap, in_ap, scale, name=""):
                """psum -> sbuf convert+scale on Act or DVE (round-robin)."""
                if evac_eng() == "A":
                    nc.scalar.activation(out=out_ap, in_=in_ap,
                                         func=Act.Identity, scale=scale)
                else:
                    nc.vector.tensor_scalar_mul(out=out_ap, in0=in_ap,
                                                scalar1=scale)

            def sc_pair(name):
                return ps_sc.tile([PC, 2, 512], f32, tag="sc", name=name)

            def layernorm(l, r, gcol, name, out_dt=f8, emit_tb=True):
                """r: [128, 2, 300] f32 -> (tT_new, tb_new[out_dt]).
                All SBUF->SBUF elementwise goes to Pool; psum reads to A/D."""
                rb = acts.tile([PC, EC, NQ], bf, tag="rb", name=f"rb{name}", bufs=1)
                nc.gpsimd.tensor_copy(out=rb, in_=r)
                tsq = acts.tile([PC, EC, NQ], bf, tag="tsq", name=f"tsq{name}", bufs=1)
                nc.gpsimd.tensor_mul(out=tsq, in0=rb, in1=rb)
                ss = ps_pp.tile([PC, 2, 512], f32, tag="pp", name=f"lns{name}")
                for c in range(EC):
                    nc.tensor.matmul(out=ss[:, 0, 0:NQ], lhsT=ones,
                                     rhs=rb[:, c, :],
                                     start=(c == 0), stop=(c == EC - 1))
                for c in range(EC):
                    nc.tensor.matmul(out=ss[:, 1, 0:NQ], lhsT=ones,
                                     rhs=tsq[:, c, :],
                                     start=(c == 0), stop=(c == EC - 1))
                mean = stp.tile([PC, NQ], f32, tag="mean", name=f"mean{name}", bufs=1)
                nc.scalar.activation(out=mean, in_=ss[:, 0, 0:NQ],
                                     func=Act.Identity, scale=1.0 / E)
                c1 = acts.tile([PC, EC, NQ], f32, tag="c1", name=f"c1{name}", bufs=1)
                nc.gpsimd.tensor_sub(out=c1, in0=r, in1=_bcmid(mean, EC))
                msq = stp.tile([PC, NQ], f32, tag="msq", name=f"msq{name}", bufs=1)
                nc.gpsimd.tensor_mul(out=msq, in0=mean, in1=mean)
                var = stp.tile([PC, NQ], f32, tag="var", name=f"var{name}", bufs=1)
                nc.vector.scalar_tensor_tensor(out=var, in0=ss[:, 1, 0:NQ],
                                               scalar=1.0 / E,
                                               in1=msq, op0=Alu.mult, op1=Alu.subtract)
                sd = stp.tile([PC, NQ], f32, tag="sd", name=f"sd{name}", bufs=1)
                nc.scalar.activation(out=sd, in_=var, func=Act.Sqrt, bias=eps[:, 0:1])
                rstd = stp.tile([PC, NQ], f32, tag="rstd", name=f"rstd{name}", bufs=1)
                nc.vector.reciprocal(out=rstd, in_=sd)
                c2 = acts.tile([PC, EC, NQ], f32, tag="c2", name=f"c2{name}", bufs=1)
                nc.gpsimd.tensor_mul(out=c2, in0=c1, in1=_bcmid(rstd, EC))
                t_new = acts.tile([PC, EC, NQ], f32, tag="tT", name=f"t{name}")
                tb_new = None
                if emit_tb:
                    tb_new = acts.tile([PC, EC, NQP if out_dt == f8 else NQ],
                                       out_dt,
                                       tag="tb8" if out_dt == f8 else "tbb",
                                       name=f"tb{name}")
                if gcol is None:
                    g, b = fin[:, 0:2], fin[:, 2:4]
                else:
                    g = sm[:, gcol:gcol + 2]
                    b = sm[:, gcol + 2:gcol + 4]
                for c in range(EC):
                    if emit_tb:
                        # critical path (feeds next matmuls): DVE
                        nc.vector.tensor_scalar(out=tb_new[:, c, 0:NQ], in0=c2[:, c, :],
                                                scalar1=g[:, c:c + 1], scalar2=b[:, c:c + 1],
                                                op0=Alu.mult, op1=Alu.add)
                    nc.gpsimd.tensor_scalar(out=t_new[:, c, :], in0=c2[:, c, :],
                                            scalar1=g[:, c:c + 1], scalar2=b[:, c:c + 1],
                                            op0=Alu.mult, op1=Alu.add)
                return t_new, tb_new

            def emit_exp(eng, sc_ap, out8_ap, out16_ap):
                """probabilities from scores: exact exp on Act (fp8 out) or
                Schraudolph int16-bitcast on DVE (bf16 out)."""
                if eng == "A":
                    nc.scalar.activation(out=out8_ap, in_=sc_ap, func=Act.Exp)
                else:
                    nc.vector.tensor_scalar(out=out16_ap.bitcast(dt.int16), in0=sc_ap,
                                            scalar1=A16, scalar2=B16,
                                            op0=Alu.mult, op1=Alu.add)

            def attention_pass(tag, l, pi, hs, groups, emit_scores, vmat, kpart,
                               attnN, av, cls_of):
                """4 head-chains; pair-granular exp on Act/DVE; flipped AV
                (lagging the scores) into one psum bank slot."""
                ng = len(groups)
                armed = [False]   # psum start=True arms the WHOLE 2KB bank for
                # zero-on-first-touch, so exactly one start per av bank: every
                # region's first write then zeroes itself, later ones accumulate.

                def flush(gi, pms):
                    for h in hs:
                        isA, pm, tl = pms[h]
                        h4 = h % 4
                        for qc, (q0, qsz) in enumerate(QCH):
                            outap = av[0:qsz, qc, VW * h4:VW * (h4 + 1)]
                            st = not armed[0]
                            armed[0] = True
                            if isA and len(tl) == 2:
                                nc.tensor.matmul(
                                    out=outap,
                                    lhsT=pm[0:kpart, 0:2, q0:q0 + qsz],
                                    rhs=vmat[0:kpart, tl[0]:tl[0] + 2, h, 0:VW],
                                    start=st, stop=(gi == ng - 1),
                                    perf_mode=DR, skip_group_check=True)
                            else:
                                for j, t in enumerate(tl):
                                    nc.tensor.matmul(
                                        out=outap,
                                        lhsT=pm[0:kpart, j, q0:q0 + qsz],
                                        rhs=vmat[0:kpart, t, h, 0:VW],
                                        start=(st and j == 0),
                                        stop=(gi == ng - 1 and j == len(tl) - 1),
                                        skip_group_check=True)

                pending = []
                for gi, tl in enumerate(groups):
                    pms = {}
                    for h in hs:
                        cls = cls_of(h, gi, len(tl))
                        p8 = probs.tile([PC, 2, NQP], f8, tag="p8",
                                        name=f"p8{tag}{l}h{h}g{gi}")
                        p16 = probs.tile([PC, 2, NQP], bf, tag="p16",
                                         name=f"p16{tag}{l}h{h}g{gi}")
                        isA = cls[0] == "A"
                        sc = sc_pair(f"sc{tag}{l}h{h}g{gi}")
                        for j, t in enumerate(tl):
                            emit_scores(h, t, sc[:, j, :])
                        n = len(tl)
                        emit_exp(cls[0], sc[0:kpart, 0:n, 0:NQ],
                                 p8[0:kpart, 0:n, 0:NQ], p16[0:kpart, 0:n, 0:NQ])
                        pms[h] = (isA, p8 if isA else p16, tl)
                    pending.append((gi, pms))
                    if len(pending) > 1:
                        flush(*pending.pop(0))
                for it in pending:
                    flush(*it)
                # normalize all 4 heads per q-chunk into attnN
                for qc, (q0, qsz) in enumerate(QCH):
                    rec = stp.tile([PC, 4, 1], f32, tag="rec",
                                   name=f"rec{tag}{l}p{pi}q{qc}", bufs=3)
                    avh = av[0:qsz, qc, :].rearrange("q (h w) -> q h w", w=VW)
                    nc.vector.reciprocal(out=rec[0:qsz, :, :], in_=avh[:, :, D:D + 1])
                    nc.vector.tensor_mul(
                        out=attnN[0:qsz, qc, pi * PC:(pi + 1) * PC]
                            .rearrange("q (h d) -> q h d", d=D),
                        in0=avh[:, :, 0:D],
                        in1=_bclast(rec[0:qsz, 0:4, 0:1], D))

            def run_attention(tag, l, emit_scores, vmat, kpart, cls_of):
                attnN = acts.tile([PC, 3, E], f32, tag="attnN",
                                  name=f"attnN{tag}{l}")
                avt = ps_pp.tile([PC, 2, 512], f32, tag="pp", name=f"av{tag}{l}")
                if tag == "sa":
                    groups = [[0, 1], [2]]
                else:
                    groups = [[2 * g, 2 * g + 1] for g in range(KT // 2)]
                for pi in range(2):
                    av = avt[:, pi, 0:3 * 4 * VW].rearrange("q (c w) -> q c w", w=4 * VW)
                    attention_pass(tag, l, pi, [4 * pi + i for i in range(4)],
                                   groups, emit_scores, vmat, kpart,
                                   attnN, av, cls_of)
                # transpose back to feature-major (f32 transpose; evac A/D)
                attn = acts.tile([PC, EC, NQ], bf, tag="attn", name=f"attn{tag}{l}")
                for fc in range(EC):
                    for qc, (q0, qsz) in enumerate(QCH):
                        pt = sc_pair(f"pt{tag}{l}f{fc}q{qc}")
                        nc.tensor.transpose(
                            out=pt[:, 0, 0:qsz],
                            in_=attnN[0:qsz, qc, fc * PC:(fc + 1) * PC],
                            identity=idb[0:qsz, 0:qsz])
                        evac_scale(attn[:, fc, q0:q0 + qsz], pt[:, 0, 0:qsz], 1.0)
                return attn

            def out_proj_residual(l, w_sb, attn, bcol, tT, name):
                r = acts.tile([PC, EC, NQ], f32, tag="r", name=f"r{name}", bufs=1)
                po = sc_pair(f"po{name}")
                for co in range(EC):
                    for ci in range(EC):
                        nc.tensor.matmul(out=po[:, co, 0:NQ],
                                         lhsT=w_sb[:, ci, PC * co:PC * (co + 1)],
                                         rhs=attn[:, ci, :],
                                         start=(ci == 0), stop=(ci == EC - 1))
                    nc.vector.scalar_tensor_tensor(
                        out=r[:, co, :], in0=po[:, co, 0:NQ],
                        scalar=sm[:, bcol + co:bcol + co + 1],
                        in1=tT[:, co, :], op0=Alu.add, op1=Alu.add)
                return r

            def emit_kproj(l, wck, name):
                """k projection into kT (fp8, already sqrt(S)-scaled weights),
                then 8 band DMAs refreshing the bd arena."""
                ppk = ps_pp.tile([PC, 2, 512], f32, tag="pp", name=f"pk{name}")
                for co in range(EC):
                    for nk in range(NKC):
                        j = (co * NKC + nk) % 2
                        nc.tensor.matmul(out=ppk[:, j, :],
                                         lhsT=wck[:, :, PC * co:PC * (co + 1)],
                                         rhs=memT[:, :, 512 * nk:512 * (nk + 1)],
                                         start=True, stop=True, perf_mode=DR)
                        evac_scale(kT[:, co, 512 * nk:512 * (nk + 1)],
                                   ppk[:, j, :], 1.0 / WS)
                for h in range(H):
                    p0 = 32 * (h % 4)
                    c0 = _bd_band(h, 0)
                    nc.sync.dma_start(
                        out=bd[p0:p0 + 32, c0:c0 + BDH]
                            .rearrange("p (t k) -> p t k", k=PC),
                        in_=kT[p0:p0 + 32, h // 4, :]
                            .rearrange("p (t k) -> p t k", k=PC))

            # ---- layer 0 k projection (prologue) ----
            wck0 = wp.tile([PC, EC, E], f8, tag="wck", name="wck0")
            nc.sync.dma_start(out=wck0, in_=r2(d_wck[0]))
            emit_kproj(0, wck0, "p0")

            sched_i = {"ca": 0, "sap": 0, "sas": 0}

            def ca_cls(h, gi, n):
                c = CA_CLS[sched_i["ca"] % len(CA_CLS)]
                sched_i["ca"] += 1
                return c

            def sa_cls(h, gi, n):
                if n == 2:
                    c = SA_CLS[sched_i["sap"] % len(SA_CLS)]
                    sched_i["sap"] += 1
                else:
                    c = SA_SGL[sched_i["sas"] % len(SA_SGL)]
                    sched_i["sas"] += 1
                return c

            for l in range(nlayers):
                # ---- layer weight loads ----
                wqk = wp.tile([PC, EC, 2 * E], f8, tag="wqk", name=f"wqk{l}")
                nc.sync.dma_start(out=wqk, in_=r2(d_wqk[l]))
                wsv = wp.tile([PC, EC, E], f8, tag="wsv", name=f"wsv{l}")
                nc.sync.dma_start(out=wsv, in_=r2(d_wsv[l]))
                wso = wp.tile([PC, EC, E], bf, tag="wso", name=f"wso{l}")
                nc.sync.dma_start(out=wso, in_=r2(d_wso[l]))
                wcq = wp.tile([PC, EC, E], f8, tag="wcq", name=f"wcq{l}")
                nc.sync.dma_start(out=wcq, in_=r2(d_wcq[l]))
                wcv = wp.tile([PC, EC, E], f8, tag="wcv", name=f"wcv{l}")
                nc.sync.dma_start(out=wcv, in_=r2(d_wcv[l]))
                wco = wp.tile([PC, EC, E], bf, tag="wco", name=f"wco{l}")
                nc.sync.dma_start(out=wco, in_=r2(d_wco[l]))
                w1 = wp.tile([PC, EC, F], bf, tag="w1", name=f"w1_{l}", bufs=1)
                nc.sync.dma_start(out=w1, in_=r2(d_w1[l]))
                w2 = wp.tile([PC, FT, E], bf, tag="w2", name=f"w2_{l}", bufs=1)
                nc.sync.dma_start(out=w2, in_=d_w2[l].rearrange("(c p) o -> p c o", p=PC))
                sm = wp.tile([PC, NS], f32, tag="sm", name=f"sm{l}")
                nc.sync.dma_start(out=sm, in_=d_sm[l])

                # ---- SA qkv projections (fp8 DoubleRow) ----
                qk_sa = acts.tile([PC, 4, NQ], f8, tag="qk_sa", name=f"qk_sa{l}")
                for cp in range(2):
                    po = sc_pair(f"pqk{l}_{cp}")
                    for j in range(2):
                        co = 2 * cp + j
                        nc.tensor.matmul(out=po[:, j, 0:NQ],
                                         lhsT=wqk[:, :, PC * co:PC * (co + 1)],
                                         rhs=tb8[:, :, 0:NQ], start=True, stop=True,
                                         perf_mode=DR)
                        if co < 2:
                            nc.vector.tensor_scalar(
                                out=qk_sa[:, co, :], in0=po[:, j, 0:NQ],
                                scalar1=1.0 / WS,
                                scalar2=sm[:, C_BQ_SA + co:C_BQ_SA + co + 1],
                                op0=Alu.mult, op1=Alu.add)
                        else:
                            evac_scale(qk_sa[:, co, :], po[:, j, 0:NQ], 1.0 / WS)
                for tt, (t0, tsz) in enumerate(TOK_TILES):
                    pv = sc_pair(f"pvsa{l}_{tt}")
                    nc.tensor.matmul(out=pv[0:tsz, 0, 0:E],
                                     lhsT=tb8[:, :, t0:t0 + tsz],
                                     rhs=wsv, start=True, stop=True, perf_mode=DR)
                    evac_scale(vsa[0:tsz, tt, :, 0:D], _hd(pv[0:tsz, 0, 0:E]),
                               1.0 / WS)

                # ---- CA v projection (hoisted: overlaps SA attention) ----
                for tp in range(KT // 2):
                    pv = sc_pair(f"pvca{l}_{tp}")
                    for j in range(2):
                        nc.tensor.matmul(
                            out=pv[:, j, 0:E],
                            lhsT=memT[:, :, PC * (2 * tp + j):PC * (2 * tp + j + 1)],
                            rhs=wcv, start=True, stop=True, perf_mode=DR)
                    evac_scale(vca[:, 2 * tp:2 * tp + 2, :, 0:D],
                               pv[:, 0:2, 0:E].rearrange("p t (h d) -> p t h d", d=D),
                               1.0 / WS)

                # ---- SA attention ----
                def sa_scores(h, t, sc1):
                    po4, ci = 32 * (h % 4), h // 4
                    t0, tsz = TOK_TILES[t]
                    nc.tensor.matmul(
                        out=sc1[0:tsz, 0:NQ],
                        lhsT=qk_sa[po4:po4 + 32, 2 + ci, t0:t0 + tsz],
                        rhs=qk_sa[po4:po4 + 32, ci, 0:NQ],
                        start=True, stop=True, tile_position=(po4, 0))

                attn = run_attention("sa", l, sa_scores, vsa, 100, sa_cls)
                if DBG and l == 0:
                    nc.sync.dma_start(out=d_dba, in_=attn)
                    nc.sync.dma_start(out=d_dbq, in_=qk_sa)
                    nc.sync.dma_start(out=d_dbv, in_=vsa)

                # ---- SA out proj + LN1 ----
                r = out_proj_residual(l, wso, attn, C_BO_SA, tT, f"so{l}")
                tT, tb8 = layernorm(l, r, C_LN, f"ln1_{l}", out_dt=f8)
                if DBG and l == 0:
                    nc.sync.dma_start(out=r2(d_dbg[0]), in_=tT)

                # ---- CA q projection into the mask arena ----
                po = sc_pair(f"pq_ca{l}")
                for co in range(EC):
                    nc.tensor.matmul(out=po[:, co, 0:NQ],
                                     lhsT=wcq[:, :, PC * co:PC * (co + 1)],
                                     rhs=tb8[:, :, 0:NQ], start=True, stop=True, perf_mode=DR)
                    nc.vector.tensor_scalar(
                        out=marena[:, QOFF + NQP * co:QOFF + NQP * co + NQ],
                        in0=po[:, co, 0:NQ],
                        scalar1=1.0 / WS,
                        scalar2=sm[:, C_BQ_CA + co:C_BQ_CA + co + 1],
                        op0=Alu.mult, op1=Alu.add)

                # ---- CA attention ----
                def ca_scores(h, t, sc1):
                    nc.tensor.matmul(
                        out=sc1[:, 0:NQ],
                        lhsT=_slot2(bd, _bd_mci(h), _bd_band(h, t), PC),
                        rhs=_slot2(marena, NQP * t, QOFF + NQP * (h // 4), NQ),
                        start=True, stop=True, perf_mode=DR)

                attn = run_attention("ca", l, ca_scores, vca, PC, ca_cls)

                # ---- CA out proj + LN2 (bf16 tb for FFN) ----
                r = out_proj_residual(l, wco, attn, C_BO_CA, tT, f"co{l}")
                tT, tb = layernorm(l, r, C_LN + 4, f"ln2_{l}", out_dt=bf)
                if DBG and l == 0:
                    nc.sync.dma_start(out=r2(d_dbg[1]), in_=tT)

                # ---- next layer k projection (overlaps with FFN) ----
                if l + 1 < nlayers:
                    wck_n = wp.tile([PC, EC, E], f8, tag="wck", name=f"wck{l + 1}")
                    nc.sync.dma_start(out=wck_n, in_=r2(d_wck[l + 1]))
                    emit_kproj(l + 1, wck_n, f"p{l + 1}")

                # ---- FFN (bf16) ----
                hT = acts.tile([PC, FT, NQ], bf, tag="hT", name=f"hT{l}", bufs=1)
                for fp_ in range(FT // 2):
                    pf = sc_pair(f"pf1_{l}_{fp_}")
                    for j in range(2):
                        ft = 2 * fp_ + j
                        for ci in range(EC):
                            nc.tensor.matmul(out=pf[:, j, 0:NQ],
                                             lhsT=w1[:, ci, PC * ft:PC * (ft + 1)],
                                             rhs=tb[:, ci, :],
                                             start=(ci == 0), stop=(ci == EC - 1))
                        eng = RELU_E[ft % len(RELU_E)]
                        if eng == "A":
                            nc.scalar.activation(
                                out=hT[:, ft, :], in_=pf[:, j, 0:NQ],
                                func=Act.Relu,
                                bias=sm[:, C_B1 + ft:C_B1 + ft + 1])
                        else:
                            nc.vector.tensor_scalar(
                                out=hT[:, ft, :], in0=pf[:, j, 0:NQ],
                                scalar1=sm[:, C_B1 + ft:C_B1 + ft + 1], scalar2=0.0,
                                op0=Alu.add, op1=Alu.max)
                r = acts.tile([PC, EC, NQ], f32, tag="r", name=f"rf{l}", bufs=1)
                p2 = sc_pair(f"pf2_{l}")
                for co in range(EC):
                    for fc in range(FT):
                        nc.tensor.matmul(out=p2[:, co, 0:NQ],
                                         lhsT=w2[:, fc, PC * co:PC * (co + 1)],
                                         rhs=hT[:, fc, :],
                                         start=(fc == 0), stop=(fc == FT - 1))
                    nc.vector.scalar_tensor_tensor(
                        out=r[:, co, :], in0=p2[:, co, 0:NQ],
                        scalar=sm[:, C_B2 + co:C_B2 + co + 1],
                        in1=tT[:, co, :], op0=Alu.add, op1=Alu.add)
                last = l + 1 == nlayers
                tT, tb8 = layernorm(l, r, C_LN + 8, f"ln3_{l}",
                                    out_dt=f8, emit_tb=not last)
                if DBG and l == 0:
                    nc.sync.dma_start(out=r2(d_dbg[2]), in_=tT)

            # ---- final LN + store ----
            outT, _ = layernorm(None, tT, None, "lnf", emit_tb=False)
            nc.sync.dma_start(out=r2(d_out), in_=outT)

    nc.compile()
    return nc


def _pack_inputs(inputs, nlayers=L):
    """Host-side layout prep: transpose / cast / fold. Returns per-core in_maps."""
    smalls = np.zeros((nlayers, PC, NS), np.float32)
    for l in range(nlayers):
        def put(col, vec):
            n = vec.shape[0] // PC
            smalls[l, :, col:col + n] = vec.reshape(n, PC).T
        sa_bq = np.asarray(inputs["sa_bqkv"][l][:E], np.float32)
        sa_bv = np.asarray(inputs["sa_bqkv"][l][2 * E:], np.float32)
        sa_wo = np.asarray(inputs["sa_wo"][l], np.float32)
        ca_bv = np.asarray(inputs["ca_bv"][l], np.float32)
        ca_wo = np.asarray(inputs["ca_wo"][l], np.float32)
        put(C_BQ_SA, RS * sa_bq)
        put(C_BO_SA, np.asarray(inputs["sa_bo"][l], np.float32) + sa_wo @ sa_bv)
        put(C_BQ_CA, RS * np.asarray(inputs["ca_bq"][l], np.float32))
        put(C_BO_CA, np.asarray(inputs["ca_bo"][l], np.float32) + ca_wo @ ca_bv)
        put(C_B1, np.asarray(inputs["f_b1"][l], np.float32))
        put(C_B2, np.asarray(inputs["f_b2"][l], np.float32))
        put(C_LN, np.asarray(inputs["ln1g"][l], np.float32))
        put(C_LN + 2, np.asarray(inputs["ln1b"][l], np.float32))
        put(C_LN + 4, np.asarray(inputs["ln2g"][l], np.float32))
        put(C_LN + 6, np.asarray(inputs["ln2b"][l], np.float32))
        put(C_LN + 8, np.asarray(inputs["ln3g"][l], np.float32))
        put(C_LN + 10, np.asarray(inputs["ln3b"][l], np.float32))
    finals = np.zeros((PC, 4), np.float32)
    finals[:, 0:2] = np.asarray(inputs["lnfg"], np.float32).reshape(2, PC).T
    finals[:, 2:4] = np.asarray(inputs["lnfb"], np.float32).reshape(2, PC).T

    def T(x):
        return np.ascontiguousarray(np.swapaxes(np.asarray(x, np.float32), -1, -2))

    shared = {
        "w_sa_qk8": (T(inputs["sa_wqkv"][:nlayers, :2 * E]) * (RS * WS)).astype(F8),
        "w_sa_v8": (T(inputs["sa_wqkv"][:nlayers, 2 * E:]) * WS).astype(F8),
        "w_sa_o": T(inputs["sa_wo"][:nlayers]).astype(BF16),
        "w_ca_q8": (T(inputs["ca_wq"][:nlayers]) * (RS * WS)).astype(F8),
        "w_ca_k8": (T(inputs["ca_wk"][:nlayers]) * (RS * WS)).astype(F8),
        "w_ca_v8": (T(inputs["ca_wv"][:nlayers]) * WS).astype(F8),
        "w_ca_o": T(inputs["ca_wo"][:nlayers]).astype(BF16),
        "w_f1": T(inputs["f_w1"][:nlayers]).astype(BF16),
        "w_f2": T(inputs["f_w2"][:nlayers]).astype(BF16),
        "smalls": smalls,
        "finals": finals,
        "idb": np.eye(PC, dtype=np.float32),
    }
    bd0 = np.zeros((PC, 2 * BDHALF), np.float32)
    bd0[:, :PC] = -CMASK * np.eye(PC, dtype=np.float32)
    bd0[:, BDHALF:BDHALF + PC] = -CMASK * np.eye(PC, dtype=np.float32)
    shared["bd0"] = bd0.astype(F8)
    vca0 = np.zeros((PC, KT, H, VW), np.float32)
    vca0[:, :, :, D:VW] = 1.0
    shared["vca0"] = vca0.astype(F8)
    vsa0 = np.zeros((PC, len(TOK_TILES), H, VW), np.float32)
    vsa0[:, :, :, D:VW] = 1.0
    shared["vsa0"] = vsa0.astype(F8)
    in_maps = []
    for b in range(B):
        m = dict(shared)
        m["tT"] = T(inputs["tgt"][b]).astype(np.float32)
        m["memT8"] = T(inputs["memory"][b]).astype(F8)
        mi = 1.0 - np.asarray(inputs["geometry_mask"][b], np.float32).T
        m["maskI8"] = np.ascontiguousarray(
            mi.reshape(KT, PC, NQ).transpose(1, 0, 2)).astype(F8)
        in_maps.append(m)
    return in_maps


_CACHE = {}


def kernel(run_opts=None, **inputs):
    nlayers = L
    if "nc" not in _CACHE:
        _CACHE["nc"] = build_nc(nlayers)
    nc = _CACHE["nc"]
    in_maps = _pack_inputs(inputs, nlayers)
    res = bass_utils.run_bass_kernel_spmd(
        nc, in_maps, core_ids=list(range(B)), **(run_opts or {}))
    _CACHE["last_result"] = res
    out = np.stack([np.asarray(r["outT"]).T for r in res.results])
    return np.ascontiguousarray(out.astype(np.float32))
